# revision 1
# baseline (speedup 1.0000x reference)
"""Trainium2 Bass kernel for GQA attention (nn_Attention_15015205667492).

Reference computation (per batch b, seq s=2048, d=2048):
  q = (x @ wq)  -> 32 heads x 64     (RoPE)
  k = (x @ wk)  ->  8 kv heads x 64  (RoPE)
  v = (x @ wv)  ->  8 kv heads x 64
  causal softmax(q k^T / 8) @ v  (GQA: kv head = q head // 4)
  out = attn @ wo

Sharding (8 cores): DP2 x TP4.
  core c: batch = c//4, head-group g = c%4 (Q heads 8g..8g+7, KV heads 2g, 2g+1).
  Each core computes attention for its 8 heads over its batch, writes the
  head-transposed attention output [512, S] bf16 to DRAM, AllGathers it within
  its 4-core batch group -> [2048, S], then computes a column slice of o_proj
  (wo[:, 512g:512(g+1)]) so per-core outputs are disjoint blocks of the final
  output (host-side unshard is pure concatenation).

Kernel layout choices:
  - x is passed transposed+bf16 (xT [d, s]) so all projections contract d on
    partitions.  Q/K come out transposed ([head-pair 128, s]) which is what
    the QK^T matmul wants as lhsT/rhs; V comes out natural ([s, kv 128]).
  - Scores are computed transposed: S^T[k, q] = kT.T @ qT per 128-k-block, so
    softmax numerator exp() runs on ScalarE and the AV matmul consumes P^T
    directly (no P transpose anywhere).
  - Softmax denominator comes free from the AV matmul: V is augmented with a
    ones column, so row 64 of the AV psum accumulates sum_k exp(s); the
    divide uses reciprocal_approx_fast (51 ULP, ample for softmax sums).
  - Causality is static: key blocks beyond the query block are skipped;
    diagonal blocks get a binary mask multiply on P^T, and far-diagonal
    blocks (j>=2) shrink the processed q-window to their visible range.
  - PSUM->SBUF drains run on ScalarE (nc.scalar.copy) to keep VectorE free;
    RoPE's rotate-half is a PE permutation matmul (host-built +-1 matrix)
    because partition-shifted DVE ops are rejected by the compiler.
"""

import sys

sys.path.insert(0, "/opt/trn_rl_repo")

import numpy as np
import ml_dtypes

N_CORES = 8
H, KVH, HD = 32, 8, 64
RG = [[0, 1, 2, 3], [4, 5, 6, 7]]

_cache = {}


def build_program(S=2048, D=2048, enable_asserts=False, NO_CC=False, bench_iters=0, phases=15, ablate=()):
    import concourse.mybir as mybir
    import concourse.tile as tile
    from concourse import bacc

    f32 = mybir.dt.float32
    bf16 = mybir.dt.bfloat16
    Exp = mybir.ActivationFunctionType.Exp

    DC = D // 128       # contraction chunks for projections
    QB = S // 512       # query blocks (512 q rows each)
    KB = S // 128       # key blocks
    DOUT = D // 4       # output column slice per core

    nc = bacc.Bacc(
        "TRN2",
        target_bir_lowering=False,
        debug=False,
        enable_asserts=enable_asserts,
        num_devices=N_CORES,
    )

    xT_d = nc.dram_tensor("xT", [D, S], bf16, kind="ExternalInput")
    wq_d = nc.dram_tensor("wq", [D, 512], bf16, kind="ExternalInput")
    wk_d = nc.dram_tensor("wk", [D, 128], bf16, kind="ExternalInput")
    wv_d = nc.dram_tensor("wv", [D, 128], bf16, kind="ExternalInput")
    wo_d = nc.dram_tensor("wo", [H * HD, DOUT], bf16, kind="ExternalInput")
    cos_d = nc.dram_tensor("cos2", [128, S], bf16, kind="ExternalInput")
    sin_d = nc.dram_tensor("sinsw2", [128, S], bf16, kind="ExternalInput")
    rot_d = nc.dram_tensor("rot", [128, 128], bf16, kind="ExternalInput")
    msk_d = nc.dram_tensor("masks", [128, 4, 1024], bf16, kind="ExternalInput")
    out_d = nc.dram_tensor("out", [S, DOUT], f32, kind="ExternalOutput")

    HC = (H * HD) // 128  # o_proj contraction chunks (16)

    with tile.TileContext(nc) as tc:
        with (
            tc.tile_pool(name="const", bufs=1) as const,
            tc.tile_pool(name="psA", bufs=2, space="PSUM") as psA,
            tc.tile_pool(name="psAV", bufs=1, space="PSUM") as psAV,
            tc.tile_pool(name="psP", bufs=2, space="PSUM") as psP,
            tc.tile_pool(name="work", bufs=2) as work,
            tc.tile_pool(name="dram", bufs=1, space="DRAM") as dram,
        ):
            # ---------------- constants / weights ----------------
            xt = []
            for i in range(DC):
                t = const.tile([128, S], bf16, name=f"xt{i}", tag=f"xt{i}")
                nc.sync.dma_start(out=t[:], in_=xT_d[128 * i : 128 * (i + 1), :])
                xt.append(t)
            wq_t = []
            for i in range(DC):
                t = const.tile([128, 512], bf16, name=f"wq{i}", tag=f"wq{i}")
                nc.sync.dma_start(out=t[:], in_=wq_d[128 * i : 128 * (i + 1), :])
                wq_t.append(t)
            wk_t = []
            wv_t = []
            for i in range(DC):
                t = const.tile([128, 128], bf16, name=f"wk{i}", tag=f"wk{i}")
                nc.sync.dma_start(out=t[:], in_=wk_d[128 * i : 128 * (i + 1), :])
                wk_t.append(t)
                t = const.tile([128, 128], bf16, name=f"wv{i}", tag=f"wv{i}")
                nc.sync.dma_start(out=t[:], in_=wv_d[128 * i : 128 * (i + 1), :])
                wv_t.append(t)
            wo_t = []
            for i in range(HC):
                t = const.tile([128, DOUT], bf16, name=f"wo{i}", tag=f"wo{i}")
                nc.sync.dma_start(out=t[:], in_=wo_d[128 * i : 128 * (i + 1), :])
                wo_t.append(t)
            cos_sb = const.tile([128, S], bf16, name="cos", tag="cos")
            nc.sync.dma_start(out=cos_sb[:], in_=cos_d[:, :])
            sin_sb = const.tile([128, S], bf16, name="sin", tag="sin")
            nc.sync.dma_start(out=sin_sb[:], in_=sin_d[:, :])
            rot_sb = const.tile([128, 128], bf16, name="rot", tag="rot")
            nc.sync.dma_start(out=rot_sb[:], in_=rot_d[:, :])
            msk_sb = const.tile([128, 4, 1024], bf16, name="msk", tag="msk")
            nc.sync.dma_start(out=msk_sb[:], in_=msk_d[:, :, :])
            ones_sb = const.tile([65, 64], f32, name="ones", tag="ones")
            nc.vector.memset(ones_sb[:], 1.0)

            def emit_body():
                # ---------------- Q/K projection + RoPE ----------------
                # RoPE in T-layout: rows = hd index (2 heads stacked), cols = seq.
                # rot-half = partition swap (0:32<->32:64, 64:96<->96:128); the
                # sign lives in the host-prepared sinsw2.
                CH = min(1024, S)
                NC2 = S // CH

                def proj_rope(w_tiles, col0, dest, c2_list=None):
                    # process in 1024-col seq chunks to bound scratch SBUF
                    for c2 in c2_list if c2_list is not None else range(NC2):
                        raw = work.tile([128, CH], bf16, name="raw", tag="raw", bufs=2)
                        tmp = work.tile([128, CH], bf16, name="ropetmp", tag="ropetmp", bufs=2)
                        for q2 in range(CH // 512):
                            qc = (CH // 512) * c2 + q2
                            pq = psP.tile([128, 512], f32, name="pq", tag="pp")
                            for dc in range(DC):
                                nc.tensor.matmul(
                                    pq[:],
                                    w_tiles[dc][:, col0 : col0 + 128],
                                    xt[dc][:, 512 * qc : 512 * (qc + 1)],
                                    start=(dc == 0),
                                    stop=(dc == DC - 1),
                                )
                            nc.scalar.copy(
                                out=raw[:, 512 * q2 : 512 * (q2 + 1)], in_=pq[:]
                            )
                        # rotate-half via PE permutation, sign folded into sinsw2
                        for q2 in range(CH // 512):
                            pr = psP.tile([128, 512], f32, name="pr", tag="pp")
                            nc.tensor.matmul(
                                pr[:],
                                rot_sb[:],
                                raw[:, 512 * q2 : 512 * (q2 + 1)],
                                start=True,
                                stop=True,
                            )
                            nc.vector.tensor_mul(
                                tmp[:, 512 * q2 : 512 * (q2 + 1)],
                                pr[:],
                                sin_sb[:, CH * c2 + 512 * q2 : CH * c2 + 512 * (q2 + 1)],
                            )
                        nc.vector.tensor_mul(
                            raw[:], raw[:], cos_sb[:, CH * c2 : CH * (c2 + 1)]
                        )
                        nc.vector.tensor_add(
                            dest[:, CH * c2 : CH * (c2 + 1)], raw[:], tmp[:]
                        )

                qT = []
                for p in range(4 if phases & 2 else 0):
                    t = const.tile([128, S], bf16, name=f"qT{p}", tag=f"qT{p}")
                    qT.append(t)
                if not (phases & 2):
                    return
                krope = work.tile([128, S], bf16, name="krope", tag="krope", bufs=1)
                proj_rope(wk_t, 0, krope)
                # duplicate each kv head across both 64-partition halves so the
                # two QK matmuls of a head pair land on disjoint PE row groups.
                kTd = []
                for h in range(2):
                    t = const.tile([128, S], bf16, name=f"kTd{h}", tag=f"kTd{h}")
                    nc.sync.dma_start(out=t[0:64, :], in_=krope[64 * h : 64 * h + 64, :])
                    nc.sync.dma_start(out=t[64:128, :], in_=krope[64 * h : 64 * h + 64, :])
                    kTd.append(t)

                # ---------------- V projection (natural layout, +ones col) -----
                v_sb = []
                for kb in range(KB if phases & 1 else 0):
                    vt = const.tile([128, 132], bf16, name=f"v{kb}", tag=f"v{kb}")
                    nc.vector.memset(vt[:, 64:65], 1.0)
                    nc.vector.memset(vt[:, 129:130], 1.0)
                    pv = psP.tile([128, 128], f32, name="pv", tag="pp")
                    for dc in range(DC):
                        nc.tensor.matmul(
                            pv[:],
                            xt[dc][:, 128 * kb : 128 * (kb + 1)],
                            wv_t[dc][:],
                            start=(dc == 0),
                            stop=(dc == DC - 1),
                        )
                    nc.vector.tensor_copy(out=vt[:, 0:64], in_=pv[:, 0:64])
                    nc.vector.tensor_copy(out=vt[:, 65:129], in_=pv[:, 64:128])
                    v_sb.append(vt)

                # ---------------- attention + AllGather + o_proj ----------------
                cc_in = [
                    dram.tile([512, 512], bf16, name=f"cin{qb}", tag=f"cin{qb}")
                    for qb in range(QB)
                ]
                cc_out = [
                    dram.tile([2048, 512], bf16, name=f"cout{qb}", tag=f"cout{qb}")
                    for qb in range(QB)
                ]

                def oproj_emit(qb):
                    cct = []
                    for hc in range(HC):
                        t = work.tile(
                            [128, 512], bf16, name=f"cct{hc}", tag=f"cct{hc}", bufs=1
                        )
                        nc.sync.dma_start(
                            out=t[:], in_=cc_out[qb][128 * hc : 128 * (hc + 1), :]
                        )
                        cct.append(t)
                    for rb in range(4):
                        po = psP.tile([128, DOUT], f32, name="po", tag="pp")
                        for hc in range(HC):
                            nc.tensor.matmul(
                                po[:],
                                cct[hc][:, 128 * rb : 128 * (rb + 1)],
                                wo_t[hc][:],
                                start=(hc == 0),
                                stop=(hc == HC - 1),
                            )
                        ot = work.tile([128, DOUT], f32, name="ot", tag="ot", bufs=1)
                        nc.scalar.copy(out=ot[:], in_=po[:])
                        nc.sync.dma_start(
                            out=out_d[
                                512 * qb + 128 * rb : 512 * qb + 128 * (rb + 1), :
                            ],
                            in_=ot[:],
                        )

                def attn_emit(qb):
                    if not (phases & 4):
                        return
                    kmax = 4 * (qb + 1)
                    for hg in range(2):  # kv head (local)
                        for p2 in range(2):  # head pair within kv group
                            pidx = 2 * hg + p2
                            pav = psAV.tile(
                                [65, 1024], f32, name="pav", tag="pav"
                            )
                            for kb in range(kmax):
                                # diagonal blocks only see queries q >= 128j:
                                # shrink the processed q-window to vw columns
                                j = kb - 4 * qb
                                vw = 512 - 128 * j if j >= 2 else 512
                                q0 = 512 * qb + (512 - vw)
                                ps = psA.tile([128, 1024], f32, name="ps", tag="ps")
                                for i in range(2):
                                    r0 = 64 * i
                                    nc.tensor.matmul(
                                        ps[:, 512 * i : 512 * i + vw],
                                        kTd[hg][r0 : r0 + 64, 128 * kb : 128 * (kb + 1)],
                                        qT[pidx][r0 : r0 + 64, q0 : q0 + vw],
                                        start=True,
                                        stop=True,
                                    )
                                pt = work.tile([128, 1024], bf16, name="pt", tag="pt", bufs=4)
                                if vw == 512:
                                    if "exp" in ablate:
                                        nc.vector.tensor_copy(out=pt[:], in_=ps[:])
                                    else:
                                        nc.scalar.activation(
                                            out=pt[:], in_=ps[:], func=Exp, scale=0.125
                                        )
                                    if j >= 0 and "mask" not in ablate:
                                        nc.vector.tensor_mul(
                                            pt[:], pt[:], msk_sb[:, j, :]
                                        )
                                else:
                                    for i in range(2):
                                        sl = slice(512 * i, 512 * i + vw)
                                        if "exp" in ablate:
                                            nc.vector.tensor_copy(
                                                out=pt[:, sl], in_=ps[:, sl]
                                            )
                                        else:
                                            nc.scalar.activation(
                                                out=pt[:, sl],
                                                in_=ps[:, sl],
                                                func=Exp,
                                                scale=0.125,
                                            )
                                        if "mask" not in ablate:
                                            # restricted tri mask == prefix of mask_0
                                            nc.vector.tensor_mul(
                                                pt[:, sl], pt[:, sl], msk_sb[:, 0, 0:vw]
                                            )
                                for i in range(2):
                                    nc.tensor.matmul(
                                        pav[:, 512 * i + 512 - vw : 512 * (i + 1)],
                                        v_sb[kb][:, 65 * hg : 65 * hg + 65],
                                        pt[:, 512 * i : 512 * i + vw],
                                        start=(kb == 0),
                                        stop=(kb == kmax - 1),
                                    )
                            # normalize: out = O^T_unnorm * (1/colsum) broadcast
                            ou = work.tile([65, 1024], f32, name="ou", tag="ou", bufs=2)
                            nc.scalar.copy(out=ou[:], in_=pav[:])
                            if "norm" in ablate:
                                for i in range(2):
                                    at = work.tile([64, 512], bf16, name="at", tag="at")
                                    nc.vector.tensor_copy(
                                        out=at[:], in_=ou[0:64, 512 * i : 512 * (i + 1)]
                                    )
                                    nc.sync.dma_start(
                                        out=cc_in[qb][
                                            128 * pidx + 64 * i : 128 * pidx + 64 * (i + 1), :
                                        ],
                                        in_=at[:],
                                    )
                                continue_norm = False
                            else:
                                continue_norm = True
                            if continue_norm:
                                rbc = work.tile([64, 1024], f32, name="rbc", tag="rbc")
                                for i in range(2):
                                    pb = psP.tile([64, 512], f32, name=f"pb{i}", tag="pp")
                                    nc.tensor.matmul(
                                        pb[:],
                                        ones_sb[64:65, :],
                                        ou[64:65, 512 * i : 512 * (i + 1)],
                                        start=True,
                                        stop=True,
                                    )
                                    nc.vector.reciprocal_approx_fast(
                                        out=rbc[:, 512 * i : 512 * (i + 1)], in_=pb[:]
                                    )
                                at = work.tile([64, 1024], bf16, name="at", tag="at")
                                nc.vector.tensor_mul(at[:], ou[0:64, :], rbc[:])
                                for i in range(2):
                                    nc.sync.dma_start(
                                        out=cc_in[qb][
                                            128 * pidx + 64 * i : 128 * pidx + 64 * (i + 1), :
                                        ],
                                        in_=at[:, 512 * i : 512 * (i + 1)],
                                    )
                    if phases & 8:
                        if NO_CC:
                            nc.sync.dma_start(
                                out=cc_out[qb][0:512, :], in_=cc_in[qb][:, :]
                            )
                        else:
                            nc.gpsimd.collective_compute(
                                "AllGather",
                                mybir.AluOpType.bypass,
                                replica_groups=RG,
                                ins=[cc_in[qb].opt()],
                                outs=[cc_out[qb].opt()],
                            )

                for c2 in range(NC2):
                    for p in range(4 if phases & 2 else 0):
                        proj_rope(wq_t, 128 * p, qT[p], c2_list=[c2])
                    for qb in range(QB):
                        if (512 * qb) // CH == c2:
                            attn_emit(qb)
                if phases & 8 and phases & 4:
                    for qb in range(QB):
                        oproj_emit(qb)

            if bench_iters:
                with tc.For_i(0, bench_iters, 1, name="bench"):
                    emit_body()
            else:
                emit_body()

    nc.compile()
    return nc


def prep_inputs(x, cos, sin, wq, wk, wv, wo):
    """Shard + reformat full inputs into per-core input maps."""
    bf = ml_dtypes.bfloat16
    b, s, d = x.shape
    dout = d // 4
    cos2 = np.tile(np.ascontiguousarray(cos.T), (2, 1)).astype(bf)
    sinT = np.ascontiguousarray(sin.T)
    sinsw = np.concatenate([-sinT[:32], sinT[32:]], axis=0)
    sinsw2 = np.tile(sinsw, (2, 1)).astype(bf)
    # rotate-half permutation: tmp[i] = raw[sigma(i)]; out = R.T @ raw
    rotm = np.zeros((128, 128), np.float32)
    for i in range(128):
        j = (i // 64) * 64 + ((i % 64) + 32) % 64
        rotm[j, i] = 1.0
    rotm = rotm.astype(bf)
    k_loc = np.arange(128)[:, None]
    q_loc = np.arange(512)[None, :]
    ms = []
    for j in range(4):
        mj = (k_loc <= q_loc - 128 * j).astype(np.float32)
        ms.append(np.concatenate([mj, mj], axis=1))
    masks = np.stack(ms, axis=1).astype(bf)  # [128, 4, 1024]

    in_maps = []
    for c in range(N_CORES):
        bb, g = divmod(c, 4)
        in_maps.append(
            {
                "xT": np.ascontiguousarray(x[bb].T).astype(bf),
                "wq": np.ascontiguousarray(wq[:, 512 * g : 512 * (g + 1)]).astype(bf),
                "wk": np.ascontiguousarray(wk[:, 128 * g : 128 * (g + 1)]).astype(bf),
                "wv": np.ascontiguousarray(wv[:, 128 * g : 128 * (g + 1)]).astype(bf),
                "wo": np.ascontiguousarray(wo[:, dout * g : dout * (g + 1)]).astype(bf),
                "cos2": cos2,
                "sinsw2": sinsw2,
                "rot": rotm,
                "masks": masks,
            }
        )
    return in_maps


def assemble_output(results, b, s, d):
    full = np.empty((b, s, d), np.float32)
    dout = d // 4
    for c in range(N_CORES):
        bb, g = divmod(c, 4)
        full[bb][:, dout * g : dout * (g + 1)] = results[c]["out"]
    return full


def kernel(**inputs):
    x = np.asarray(inputs["x"], np.float32)
    b, s, d = x.shape
    key = (s, d)
    if key not in _cache:
        _cache[key] = build_program(S=s, D=d)
    nc = _cache[key]
    in_maps = prep_inputs(
        x,
        np.asarray(inputs["cos"], np.float32),
        np.asarray(inputs["sin"], np.float32),
        np.asarray(inputs["wq"], np.float32),
        np.asarray(inputs["wk"], np.float32),
        np.asarray(inputs["wv"], np.float32),
        np.asarray(inputs["wo"], np.float32),
    )
    from concourse.bass_utils import run_bass_kernel_spmd

    res = run_bass_kernel_spmd(nc, in_maps, core_ids=list(range(N_CORES)))
    return assemble_output(res.results, b, s, d)



# revision 24
# speedup vs baseline: 1.6109x; 1.6109x over previous
"""Trainium2 Bass kernel for GQA attention (nn_Attention_15015205667492).

Reference computation (per batch b, seq s=2048, d=2048):
  q = (x @ wq)  -> 32 heads x 64     (RoPE)
  k = (x @ wk)  ->  8 kv heads x 64  (RoPE)
  v = (x @ wv)  ->  8 kv heads x 64
  causal softmax(q k^T / 8) @ v  (GQA: kv head = q head // 4)
  out = attn @ wo
Sharding (8 cores): DP2 x TP4.
  core c: batch = c//4, head-group g = c%4 (Q heads 8g..8g+7, KV heads 2g, 2g+1).
  Per-qb attention output (head-transposed, bf16) is AllGathered within the
  4-core batch group; each core then computes a 512-column slice of o_proj so
  per-core outputs are disjoint blocks of the final output.

Schedule: 4 rounds, one per 512-column sequence chunk.  Round c streams x
stage c+1 (Pool-queue DMA, 25ns issue vs 565ns on SP), projects K/V/Q for
chunk c, runs attention for query block c, and interleaves o_proj matmuls of
block c-2 into the attention stream (fills PE gaps while ScalarE computes
exp; AV matmuls lag QK by 2 blocks to hide the exp latency).  V is projected
transposed and flipped per 128-block with PE transpose matmuls (~5x fewer PE
instructions than the natural-layout projection).  Scores stay transposed
(S^T[k,q]); the AV psum's 65th row (ones column in V) accumulates softmax
denominators, inverted with reciprocal_approx_fast and spread with a gpsimd
partition_broadcast.  Drains run on Pool/ScalarE to keep VectorE for
masks+RoPE; output is written bf16 and widened on the host.
"""

import sys

sys.path.insert(0, "/opt/trn_rl_repo")

import numpy as np
import ml_dtypes

N_CORES = 8
H, KVH, HD = 32, 8, 64
RG = [[0, 1, 2, 3], [4, 5, 6, 7]]

_cache = {}


def build_program(S=2048, D=2048, enable_asserts=False, NO_CC=False, bench_iters=0,
                  phases=None, ablate=()):
    import concourse.mybir as mybir
    import concourse.tile as tile
    from concourse import bacc

    f32 = mybir.dt.float32
    bf16 = mybir.dt.bfloat16
    Exp = mybir.ActivationFunctionType.Exp

    DC = D // 128         # contraction chunks for projections (16)
    CS = 512              # seq chunk = query block
    NCH = S // CS         # rounds (4)
    DOUT = D // 4         # output column slice per core (512)
    HC = (H * HD) // 128  # o_proj contraction chunks (16)
    KB = S // 128         # key blocks (16)

    nc = bacc.Bacc(
        "TRN2",
        target_bir_lowering=False,
        debug=False,
        enable_asserts=enable_asserts,
        num_devices=N_CORES,
    )

    xT_d = nc.dram_tensor("xT", [D, S], bf16, kind="ExternalInput")
    wk_d = nc.dram_tensor("wkt", [128, DC * 128], bf16, kind="ExternalInput")
    wv_d = nc.dram_tensor("wvt", [128, DC * 128], bf16, kind="ExternalInput")
    wq_d = nc.dram_tensor("wqt", [128, DC * 512], bf16, kind="ExternalInput")
    wo_d = nc.dram_tensor("wot", [128, HC * DOUT], bf16, kind="ExternalInput")
    cos_d = nc.dram_tensor("cos2", [128, S], bf16, kind="ExternalInput")
    sin_d = nc.dram_tensor("sinsw2", [128, S], bf16, kind="ExternalInput")
    rot_d = nc.dram_tensor("rot", [128, 128], bf16, kind="ExternalInput")
    idn_d = nc.dram_tensor("ident", [128, 128], bf16, kind="ExternalInput")
    msk_d = nc.dram_tensor("masks", [128, 4, 1024], bf16, kind="ExternalInput")
    out_d = nc.dram_tensor("out", [S, DOUT], bf16, kind="ExternalOutput")

    with tile.TileContext(nc) as tc:
        with (
            tc.tile_pool(name="const", bufs=1) as const,
            tc.tile_pool(name="stream", bufs=2) as stream,
            tc.tile_pool(name="work", bufs=2) as work,
            tc.tile_pool(name="psA", bufs=2, space="PSUM") as psA,
            tc.tile_pool(name="psAV", bufs=1, space="PSUM") as psAV,
            tc.tile_pool(name="psO", bufs=1, space="PSUM") as psO,
            tc.tile_pool(name="psT", bufs=1, space="PSUM") as psT,
            tc.tile_pool(name="dram", bufs=1, space="DRAM") as dram,
        ):
            # ---------------- constants (order = SP-queue DMA order) --------
            wk_sb = const.tile([128, DC, 128], bf16, name="wk", tag="wk")
            nc.sync.dma_start(out=wk_sb[:], in_=wk_d[:, :])
            wv_sb = const.tile([128, DC, 128], bf16, name="wv", tag="wv")
            cos_sb = const.tile([128, S], bf16, name="cos", tag="cos")
            sin_sb = const.tile([128, S], bf16, name="sin", tag="sin")
            rot_sb = const.tile([128, 128], bf16, name="rot", tag="rot")
            idn_sb = const.tile([128, 128], bf16, name="idn", tag="idn")
            wq_sb = const.tile([128, DC, 512], bf16, name="wq", tag="wq")
            msk_sb = const.tile([128, 4, 1024], bf16, name="msk", tag="msk")
            wo_sb = const.tile([128, HC, DOUT], bf16, name="wo", tag="wo")

            ones_sb = const.tile([65, 64], bf16, name="ones", tag="ones")
            nc.vector.memset(ones_sb[:], 1.0)
            kTd = [
                const.tile([128, S], bf16, name=f"kTd{h}", tag=f"kTd{h}")
                for h in range(2)
            ]
            v_sb = []
            for kb in range(KB):
                vt = const.tile([128, 132], bf16, name=f"v{kb}", tag=f"v{kb}")
                nc.vector.memset(vt[:, 64:65], 1.0)
                nc.vector.memset(vt[:, 129:130], 1.0)
                v_sb.append(vt)

            # per-(block, pair) collective buffers: pair p's AllGather output
            # rows 128*g hold global head-pair hc = 4*g + p
            cc_in = [
                [dram.tile([128, 512], bf16, name=f"cin{qb}_{p}",
                           tag=f"cin{qb}_{p}") for p in range(4)]
                for qb in range(NCH)
            ]
            cc_out = [
                [dram.tile([512, 512], bf16, name=f"cout{qb}_{p}",
                           tag=f"cout{qb}_{p}") for p in range(4)]
                for qb in range(NCH)
            ]

            def emit_body():
                xts = [[None] * DC for _ in range(NCH)]
                pstate = {"n": 0, "tile": None}

                def proj_ps():
                    h = pstate["n"] % 2
                    if h == 0:
                        pstate["tile"] = psA.tile([128, 1024], f32, name="pjp",
                                                  tag="ps")
                    pstate["n"] += 1
                    return pstate["tile"][:, 512 * h: 512 * (h + 1)]

                def load_stage(st):
                    for dc in range(DC):
                        t = stream.tile([128, CS], bf16, name=f"x{dc}",
                                        tag=f"x{dc}", bufs=2)
                        nc.sync.dma_start(
                            out=t[:],
                            in_=xT_d[128 * dc: 128 * (dc + 1),
                                     CS * st: CS * (st + 1)],
                        )
                        xts[st][dc] = t

                def proj_rope(c, w_ap_fn, dest_fn):
                    """One 512-col projection chunk + RoPE; dest_fn(raw, tmp)
                    emits the final add(s)."""
                    raw = work.tile([128, CS], bf16, name="raw", tag="raw", bufs=2)
                    tmp = work.tile([128, CS], bf16, name="tmp", tag="tmp", bufs=2)
                    pq = proj_ps()
                    for dc in range(DC):
                        nc.tensor.matmul(
                            pq, w_ap_fn(dc), xts[c][dc][:],
                            start=(dc == 0), stop=(dc == DC - 1),
                        )
                    nc.scalar.copy(out=raw[:], in_=pq)
                    pr = proj_ps()
                    nc.tensor.matmul(pr, rot_sb[:], raw[:],
                                     start=True, stop=True)
                    nc.vector.tensor_mul(
                        tmp[:], pr, sin_sb[:, CS * c: CS * (c + 1)]
                    )
                    nc.vector.tensor_mul(
                        raw[:], raw[:], cos_sb[:, CS * c: CS * (c + 1)]
                    )
                    dest_fn(raw, tmp)

                def k_chunk(c):
                    sl = slice(CS * c, CS * (c + 1))

                    def dest(raw, tmp):
                        # aligned halves direct; shifted halves via SBUF DMA
                        nc.vector.tensor_add(
                            kTd[0][0:64, sl], raw[0:64, :], tmp[0:64, :]
                        )
                        nc.vector.tensor_add(
                            kTd[1][64:128, sl], raw[64:128, :], tmp[64:128, :]
                        )
                        nc.sync.dma_start(
                            out=kTd[0][64:128, sl], in_=kTd[0][0:64, sl]
                        )
                        nc.sync.dma_start(
                            out=kTd[1][0:64, sl], in_=kTd[1][64:128, sl]
                        )

                    proj_rope(c, lambda dc: wk_sb[:, dc, :], dest)

                def v_chunk(c):
                    vTc = work.tile([128, CS], bf16, name="vT", tag="vT", bufs=2)
                    pq = proj_ps()
                    for dc in range(DC):
                        nc.tensor.matmul(
                            pq, wv_sb[:, dc, :], xts[c][dc][:],
                            start=(dc == 0), stop=(dc == DC - 1),
                        )
                    nc.scalar.copy(out=vTc[:], in_=pq)
                    for k4 in range(4):
                        kb = 4 * c + k4
                        ptr = psT.tile([128, 1024], bf16, name="ptr", tag="ptr")
                        nc.tensor.matmul(
                            ptr[:, 0:128], vTc[:, 128 * k4: 128 * (k4 + 1)],
                            idn_sb[:], start=True, stop=True, is_transpose=True,
                        )
                        nc.vector.tensor_copy(
                            out=v_sb[kb][:, 0:64], in_=ptr[:, 0:64]
                        )
                        nc.vector.tensor_copy(
                            out=v_sb[kb][:, 65:129], in_=ptr[:, 64:128]
                        )

                def q_chunk(c, pidx):
                    qt = stream.tile([128, CS], bf16, name=f"qT{pidx}",
                                     tag=f"qT{pidx}", bufs=2)

                    def dest(raw, tmp):
                        nc.vector.tensor_add(qt[:], raw[:], tmp[:])

                    proj_rope(
                        c, lambda dc: wq_sb[:, dc, 128 * pidx: 128 * (pidx + 1)],
                        dest,
                    )
                    return qt

                def oproj_store(qb, rb, po):
                    otb = work.tile([128, DOUT], bf16, name="otb",
                                    tag="otb", bufs=2)
                    nc.vector.tensor_copy(out=otb[:], in_=po[:])
                    nc.sync.dma_start(
                        out=out_d[
                            CS * qb + 128 * rb: CS * qb + 128 * (rb + 1), :
                        ],
                        in_=otb[:],
                    )

                def oproj_steps(qb, cctp):
                    """Yield closures, each emitting one o_proj unit for query
                    block qb (psum alloc / matmul / drain+store per rb).
                    Global pair hc lives in cctp[hc % 4] at dim1 index hc//4."""
                    for rb in range(4):
                        po = [None]

                        def start_rb(po=po):
                            po[0] = psO.tile([128, DOUT], f32, name="po", tag="po")

                        yield start_rb
                        for hc in range(HC):
                            def mm(rb=rb, hc=hc, po=po):
                                nc.tensor.matmul(
                                    po[0][:],
                                    cctp[hc % 4][:, hc // 4,
                                                 128 * rb: 128 * (rb + 1)],
                                    wo_sb[:, hc, :],
                                    start=(hc == 0), stop=(hc == HC - 1),
                                )

                            yield mm

                        def finish_rb(qb=qb, rb=rb, po=po):
                            oproj_store(qb, rb, po[0])

                        yield finish_rb

                def oproj_tail(qb, cctp):
                    """Pair-major o_proj for the last block: accumulate pairs
                    0-2 as their AllGathers land (into psums living in the
                    freed attention psA slots), then finish pair 3 rb-by-rb so
                    stores overlap the remaining matmuls."""
                    po = [proj_ps() for _ in range(4)]
                    for p in range(3):
                        for rb in range(4):
                            for g in range(4):
                                nc.tensor.matmul(
                                    po[rb],
                                    cctp[p][:, g, 128 * rb: 128 * (rb + 1)],
                                    wo_sb[:, 4 * g + p, :],
                                    start=(p == 0 and g == 0), stop=False,
                                )
                    for rb in range(4):
                        for g in range(4):
                            nc.tensor.matmul(
                                po[rb],
                                cctp[3][:, g, 128 * rb: 128 * (rb + 1)],
                                wo_sb[:, 4 * g + 3, :],
                                start=False, stop=(g == 3),
                            )
                        oproj_store(qb, rb, po[rb])

                def attn_round(c, qts, filler):
                    kmax = 4 * (c + 1)
                    cctp = []

                    def fill(n):
                        for _ in range(n):
                            f = next(filler, None)
                            if f is None:
                                return
                            f()

                    for pidx in range(4):
                        hg = pidx // 2
                        qt = qts[pidx]
                        pav = psAV.tile([65, 1024], f32, name="pav", tag="pav")
                        pending = []

                        def emit_av(kb, vw, pt, pav=pav, kmax=kmax):
                            for i in range(2):
                                nc.tensor.matmul(
                                    pav[:, 512 * i + 512 - vw: 512 * (i + 1)],
                                    v_sb[kb][:, 65 * hg: 65 * hg + 65],
                                    pt[:, 512 * i: 512 * i + vw],
                                    start=(kb == 0), stop=(kb == kmax - 1),
                                )

                        fill(1)
                        for kb in range(kmax):
                            j = kb - 4 * c
                            vw = 512 - 128 * j if j >= 2 else 512
                            ps = psA.tile([128, 1024], f32, name="ps", tag="ps")
                            for i in range(2):
                                r0 = 64 * i
                                nc.tensor.matmul(
                                    ps[:, 512 * i: 512 * i + vw],
                                    kTd[hg][r0: r0 + 64, 128 * kb: 128 * (kb + 1)],
                                    qt[r0: r0 + 64, 512 - vw: 512],
                                    start=True, stop=True,
                                )
                            pt = work.tile([128, 1024], bf16, name="pt", tag="pt",
                                           bufs=4)
                            if vw == 512:
                                nc.scalar.activation(
                                    out=pt[:], in_=ps[:], func=Exp, scale=0.125
                                )
                                if j >= 0:
                                    nc.vector.tensor_mul(
                                        pt[:], pt[:], msk_sb[:, j, :]
                                    )
                            else:
                                for i in range(2):
                                    sl = slice(512 * i, 512 * i + vw)
                                    nc.scalar.activation(
                                        out=pt[:, sl], in_=ps[:, sl], func=Exp,
                                        scale=0.125,
                                    )
                                    nc.vector.tensor_mul(
                                        pt[:, sl], pt[:, sl], msk_sb[:, 0, 0:vw]
                                    )
                            pending.append((kb, vw, pt))
                            if len(pending) > 2:
                                emit_av(*pending.pop(0))
                            fill(1)
                        while pending:
                            emit_av(*pending.pop(0))
                        fill(4)
                        # drain AV psum fast (frees pav for the next pair):
                        # value rows via DVE copy, sum row via bf16 copy; then
                        # PE-broadcast the sums, reciprocal, scale.
                        pavc = work.tile([65, 1024], f32, name="pavc", tag="pavc",
                                         bufs=2)
                        nc.vector.tensor_copy(out=pavc[0:64, :], in_=pav[0:64, :])
                        sumb = work.tile([65, 1024], bf16, name="sumb", tag="sumb",
                                         bufs=2)
                        nc.vector.tensor_copy(out=sumb[64:65, :], in_=pav[64:65, :])
                        rbc = work.tile([64, 1024], f32, name="rbc", tag="rbc",
                                        bufs=2)
                        for i in range(2):
                            pbt = psT.tile([128, 1024], bf16, name="pb", tag="ptr")
                            pb = pbt[0:64, 0:1024].bitcast(f32)
                            nc.tensor.matmul(
                                pb, ones_sb[64:65, :],
                                sumb[64:65, 512 * i: 512 * (i + 1)],
                                start=True, stop=True,
                            )
                            nc.vector.reciprocal_approx_fast(
                                out=rbc[:, 512 * i: 512 * (i + 1)], in_=pb
                            )
                        at = work.tile([64, 1024], bf16, name="at", tag="at",
                                       bufs=2)
                        nc.vector.tensor_mul(at[:], pavc[0:64, :], rbc[:])
                        # at[p, 512i+col] -> cc_in rows 64i+p
                        nc.sync.dma_start(
                            out=cc_in[c][pidx][:, :].rearrange(
                                "(i p) col -> p i col", i=2
                            ),
                            in_=at[:, :].rearrange("p (i col) -> p i col", i=2),
                        )
                        if NO_CC:
                            nc.sync.dma_start(
                                out=cc_out[c][pidx][0:128, :],
                                in_=cc_in[c][pidx][:, :],
                            )
                        else:
                            nc.gpsimd.collective_compute(
                                "AllGather",
                                mybir.AluOpType.bypass,
                                replica_groups=RG,
                                ins=[cc_in[c][pidx].opt()],
                                outs=[cc_out[c][pidx].opt()],
                            )
                        cp = work.tile([128, 4, 512], bf16, name=f"cct{pidx}",
                                       tag=f"cct{pidx}", bufs=2)
                        nc.sync.dma_start(
                            out=cp[:],
                            in_=cc_out[c][pidx][:, :].rearrange(
                                "(g p) col -> p g col", p=128
                            ),
                        )
                        cctp.append(cp)
                    fill(100)
                    return cctp

                # ---------------- rounds ----------------
                load_stage(0)
                nc.sync.dma_start(out=cos_sb[:], in_=cos_d[:, :])
                nc.sync.dma_start(out=sin_sb[:], in_=sin_d[:, :])
                nc.sync.dma_start(out=wv_sb[:], in_=wv_d[:, :])
                nc.sync.dma_start(out=rot_sb[:], in_=rot_d[:, :])
                nc.sync.dma_start(out=idn_sb[:], in_=idn_d[:, :])
                nc.sync.dma_start(out=wq_sb[:], in_=wq_d[:, :])
                load_stage(1)
                nc.sync.dma_start(out=msk_sb[:], in_=msk_d[:, :, :])
                nc.sync.dma_start(out=wo_sb[:], in_=wo_d[:, :])
                ccts = {}
                for c in range(NCH):
                    if 2 <= c + 1 < NCH:
                        load_stage(c + 1)
                    k_chunk(c)
                    v_chunk(c)
                    qts = [q_chunk(c, p) for p in range(4)]
                    filler = oproj_steps(c - 1, ccts[c - 1]) if c >= 1 else iter(())
                    ccts[c] = attn_round(c, qts, filler)
                oproj_tail(NCH - 1, ccts[NCH - 1])

            if bench_iters:
                with tc.For_i(0, bench_iters, 1, name="bench"):
                    emit_body()
            else:
                emit_body()

    nc.compile()
    return nc


def prep_inputs(x, cos, sin, wq, wk, wv, wo):
    """Shard + reformat full inputs into per-core input maps."""
    bf = ml_dtypes.bfloat16
    b, s, d = x.shape
    dout = d // 4
    dc = d // 128
    cos2 = np.tile(np.ascontiguousarray(cos.T), (2, 1)).astype(bf)
    sinT = np.ascontiguousarray(sin.T)
    sinsw = np.concatenate([-sinT[:32], sinT[32:]], axis=0)
    sinsw2 = np.tile(sinsw, (2, 1)).astype(bf)
    # rotate-half permutation: tmp[i] = raw[sigma(i)]; out = R.T @ raw
    rotm = np.zeros((128, 128), np.float32)
    for i in range(128):
        j = (i // 64) * 64 + ((i % 64) + 32) % 64
        rotm[j, i] = 1.0
    rotm = rotm.astype(bf)
    ident = np.eye(128, dtype=np.float32).astype(bf)
    k_loc = np.arange(128)[:, None]
    q_loc = np.arange(512)[None, :]
    ms = []
    for j in range(4):
        mj = (k_loc <= q_loc - 128 * j).astype(np.float32)
        ms.append(np.concatenate([mj, mj], axis=1))
    masks = np.stack(ms, axis=1).astype(bf)  # [128, 4, 1024]

    def pack_w(w):  # [d, cols] -> [128, dc*cols] with w rows 128-blocked
        cols = w.shape[1]
        return np.ascontiguousarray(
            w.reshape(dc, 128, cols).transpose(1, 0, 2).reshape(128, dc * cols)
        ).astype(bf)

    in_maps = []
    for c in range(N_CORES):
        bb, g = divmod(c, 4)
        in_maps.append(
            {
                "xT": np.ascontiguousarray(x[bb].T).astype(bf),
                "wqt": pack_w(wq[:, 512 * g: 512 * (g + 1)]),
                "wkt": pack_w(wk[:, 128 * g: 128 * (g + 1)]),
                "wvt": pack_w(wv[:, 128 * g: 128 * (g + 1)]),
                "wot": pack_w(wo[:, dout * g: dout * (g + 1)]),
                "cos2": cos2,
                "sinsw2": sinsw2,
                "rot": rotm,
                "ident": ident,
                "masks": masks,
            }
        )
    return in_maps


def assemble_output(results, b, s, d):
    full = np.empty((b, s, d), np.float32)
    dout = d // 4
    for c in range(N_CORES):
        bb, g = divmod(c, 4)
        full[bb][:, dout * g: dout * (g + 1)] = results[c]["out"].astype(np.float32)
    return full


def kernel(**inputs):
    x = np.asarray(inputs["x"], np.float32)
    b, s, d = x.shape
    key = (s, d)
    if key not in _cache:
        _cache[key] = build_program(S=s, D=d)
    nc = _cache[key]
    in_maps = prep_inputs(
        x,
        np.asarray(inputs["cos"], np.float32),
        np.asarray(inputs["sin"], np.float32),
        np.asarray(inputs["wq"], np.float32),
        np.asarray(inputs["wk"], np.float32),
        np.asarray(inputs["wv"], np.float32),
        np.asarray(inputs["wo"], np.float32),
    )
    from concourse.bass_utils import run_bass_kernel_spmd

    res = run_bass_kernel_spmd(nc, in_maps, core_ids=list(range(N_CORES)))
    return assemble_output(res.results, b, s, d)
